# revision 35
# baseline (speedup 1.0000x reference)
"""Trainium2 Bass/Tile kernel for nn_Decoder (GRU decoder with teacher forcing).

Math (per reference):
  zx  = [enc_h_feat, z]                    (B, 1056)
  h0  = zx @ W_dh.T + b_dh                 (B, 128)
  a0  = last_obs @ W_vel.T + b_vel         (B, 2)
  rel = (sg - last_obs[:, :2]) / dt        (B, 2)
  a_t = a0 if t==0 else fut_traj[t-1,:,2:4]
  x_t = [zx, a_t, rel]  -> GRUCell(x_t, h) -> mu_t, std_t

Device strategy (8 cores, batch-sharded, 2048 rows/core):
  - Feature-on-partition, batch-on-free layout; free chunks of 512.
  - Setup: [gi_r|gi_z|gi_n|h0](512 rows) = W_big.T @ XT with K=1065
    host-packed rows [zxT; sgT; loT; ones] streamed as bf16 (halves the
    startup DMA); rel term and input-side biases folded into W_big.
    Emission is K-outer over nci pairs (8 live PSUM groups) so each xt
    K-chunk is consumed the moment its DMA lands; the short 41-row chunk
    is transferred first so the TensorEngine starts ~1us earlier.
  - Recurrence runs software-pipelined over chunk slots s=(t,c). Per slot:
      psum_r / psum_z = Whh @ h + K3 @ a3 + I @ gi   [separate banks so the
                        sigmoid read releases each bank independently]
      rz       = sigmoid(psum_rz)              [ScalarE, direct from PSUM]
      psum_hn  = Whh_n @ h                     [bank shared with gin group]
      q        = (psum_hn + b_hh_n) * r        [DVE scalar_tensor_tensor]
      u        = z * h                         [GPSIMD; needs only sigmoid]
    one slot later (gin group reuses the hn bank; hn is dead after q):
      psum_gin = I @ gi_n + K2 @ a + I @ q     [PE small-K + identity adds]
      n        = tanh(psum_gin)                [ScalarE]
      v = (z-1)*n  [DVE stt];  h' = u - v      [DVE]
    three slots later (h' chain fully drained, no PE stall):
      head: matmul with scattered-column lhsT accumulates mu/std
      pre-activations for ALL steps into 4 persistent PSUM banks.
    The lags give every cross-engine dependency a full slot of slack, so
    the TensorEngine streams its 11 matmuls/slot back-to-back.
  - Tail: per-chunk head drain -> mu = hh + b_mu [DVE], std = Exp(0.5*x +
    0.5*b_std) [ScalarE, table preloaded], per-chunk output DMAs.
  - DMA discipline: (w, xt) pairs per K-chunk in consumption order; a3 in
    6 grouped transfers on sync/scalar queues only (gpsimd-queue DMAs
    would occupy the Pool engine with SWDGE descriptor generation).
Host does only sharding/packing (a0 is a (B,6)@(6,2) matmul on host).
"""

import numpy as np
import ml_dtypes

import concourse.bass as bass
import concourse.mybir as mybir
import concourse.tile as tile
from concourse import bacc
from concourse.bass_utils import run_bass_kernel_spmd

F32 = mybir.dt.float32
F32R = mybir.dt.float32r
BF16 = mybir.dt.bfloat16
AF = mybir.ActivationFunctionType
OP = mybir.AluOpType

B, T, MLP, ZD, H, NS, NP = 16384, 24, 1024, 32, 128, 6, 2
NCORES = 8
BC = B // NCORES            # 2048 rows per core
F = 512                     # free-dim chunk
NF = BC // F                # 4 chunks
KIN = MLP + ZD + NP + NS + 1  # 1065 = zx(1056) + sg(2) + lo(6) + ones(1)
NKC = (KIN + 127) // 128    # 9 K-chunks (8x128 + 41)
DT_CONST = 0.4 * 12
A3G = 4                     # steps per a3 DMA group
NA3G = T // A3G


def build_nc(debug=False, t_steps=T):
    RD = F32R                           # matmul-operand dtype for recurrence
    GD = F32                            # gate/elementwise dtype
    nc = bacc.Bacc("TRN2", target_bir_lowering=False, debug=debug)

    # ---- DRAM I/O ----
    xt_d = nc.dram_tensor("xt", [KIN, BC], BF16, kind="ExternalInput").ap()
    wbig_d = nc.dram_tensor("wbig", [KIN, 512], BF16, kind="ExternalInput").ap()
    a3_d = nc.dram_tensor("a3", [3, t_steps * BC], BF16, kind="ExternalInput").ap()
    whht_d = nc.dram_tensor("whht", [H, 3 * H], F32, kind="ExternalInput").ap()
    k3rz_d = nc.dram_tensor("k3rz", [3, 2 * H], BF16, kind="ExternalInput").ap()
    k2n_d = nc.dram_tensor("k2n", [2, H], BF16, kind="ExternalInput").ap()
    _std_off = ((2 * t_steps + 31) // 32) * 32
    _m_head = _std_off + 2 * t_steps
    wmsx_d = nc.dram_tensor("wmsx", [H, t_steps * _m_head], F32,
                            kind="ExternalInput").ap()
    id_d = nc.dram_tensor("ident128", [H, H], F32, kind="ExternalInput").ap()
    bhhn_d = nc.dram_tensor("bhhn", [H, 1], F32, kind="ExternalInput").ap()
    bmu_d = nc.dram_tensor("bmu48", [2 * t_steps, 1], F32, kind="ExternalInput").ap()
    bstd_d = nc.dram_tensor("bstd48", [2 * t_steps, 1], F32, kind="ExternalInput").ap()
    omu_d = nc.dram_tensor("omu", [2 * t_steps, BC], F32, kind="ExternalOutput").ap()
    ostd_d = nc.dram_tensor("ostd", [2 * t_steps, BC], F32, kind="ExternalOutput").ap()

    with tile.TileContext(nc) as tc:
        with tc.tile_pool(name="persist", bufs=1) as pp:
            # persistent SBUF state
            gi_r = pp.tile([H, BC], RD)
            gi_z = pp.tile([H, BC], RD)
            gi_n = pp.tile([H, BC], RD)
            hA = pp.tile([H, BC], RD)
            hB = pp.tile([H, BC], RD)
            # head accumulator rows: [mu0 xT | mu1 xT | pad | std0 xT | std1 xT]
            std_off = ((2 * t_steps + 31) // 32) * 32
            m_head = std_off + 2 * t_steps
            hh = pp.tile([m_head, BC], F32, name="headacc")
            whht_t = pp.tile([H, 3 * H], RD)
            k3rz_t = pp.tile([3, 2 * H], BF16)
            k2n_t = pp.tile([2, H], BF16)
            wmsx_t = pp.tile([H, t_steps * m_head], RD)
            bhhn_t = pp.tile([H, 1], F32)
            bmu_t = pp.tile([2 * t_steps, 1], F32)
            bstd_t = pp.tile([2 * t_steps, 1], F32)
            ident = pp.tile([H, H], RD)

            gi_dst = [gi_r, gi_z, gi_n, None]

            # ---- setup: [gi | h0] = W_big.T @ XT  (bf16 -> fp32 PSUM) ----
            with tc.tile_pool(name="xtp", bufs=1) as xtp, \
                 tc.tile_pool(name="wp", bufs=1) as wp, \
                 tc.tile_pool(name="sps", bufs=1, space="PSUM") as sps:
                # weights FIRST (small; the first matmul group needs them),
                # then xt in 9 large per-K-chunk transfers
                # per-K-chunk (w, xt) pairs in consumption order; everything
                # only needed once the loop runs (whht, heads, biases) after
                w_tiles, xt_tiles = [], []
                KORD = [NKC - 1] + list(range(NKC - 1))  # small chunk first
                for ki, k in enumerate(KORD):
                    kc = min(128, KIN - 128 * k)
                    qq = [nc.sync, nc.scalar, nc.gpsimd][ki % 3]
                    w_k = wp.tile([kc, 512], BF16, name=f"w{k}", tag=f"w{k}")
                    qq.dma_start(w_k[:], wbig_d[128 * k:128 * k + kc])
                    xt_k = xtp.tile([kc, BC], BF16, name=f"xt{k}", tag=f"xt{k}")
                    qq.dma_start(xt_k[:], xt_d[128 * k:128 * k + kc])
                    w_tiles.append(w_k)
                    xt_tiles.append(xt_k)
                nc.sync.dma_start(whht_t[:], whht_d.bitcast(RD))
                nc.scalar.dma_start(k3rz_t[:], k3rz_d)
                nc.gpsimd.dma_start(ident[:], id_d.bitcast(RD))
                nc.scalar.dma_start(k2n_t[:], k2n_d)
                nc.gpsimd.dma_start(wmsx_t[:], wmsx_d.bitcast(RD))
                nc.sync.dma_start(bhhn_t[:], bhhn_d)
                nc.sync.dma_start(bmu_t[:], bmu_d)
                nc.sync.dma_start(bstd_t[:], bstd_d)

                # k-outer over nci pairs: each xt K-chunk is consumed as it
                # lands (8 live PSUM groups = 2 nci x 4 m); emission order
                # matches data-arrival order so the PE FIFO never head-of-line
                # blocks on a late DMA.
                MORD = (3, 0, 1, 2)
                for np0 in range(0, NF, 2):
                    pss = {}
                    for nci in (np0, np0 + 1):
                        for m in MORD:
                            pss[(nci, m)] = sps.tile(
                                [128, F], F32, name=f"setps{nci}_{m}",
                                tag=f"setps{nci & 1}_{m}")
                    for ki in range(NKC):
                        for nci in (np0, np0 + 1):
                            for m in MORD:
                                nc.tensor.matmul(
                                    pss[(nci, m)][:],
                                    w_tiles[ki][:, 128 * m:128 * (m + 1)],
                                    xt_tiles[ki][:, nci * F:(nci + 1) * F],
                                    start=(ki == 0), stop=(ki == NKC - 1),
                                    skip_group_check=True,
                                )
                    for nci in (np0, np0 + 1):
                        for m in MORD:
                            dst = gi_dst[m] if m < 3 else hA
                            if (m + nci) % 2 == 0:
                                nc.vector.tensor_copy(
                                    dst[:, nci * F:(nci + 1) * F],
                                    pss[(nci, m)][:])
                            else:
                                nc.scalar.copy(dst[:, nci * F:(nci + 1) * F],
                                               pss[(nci, m)][:])

            # ---- recurrence (software-pipelined over chunk slots) ----
            # Slot s = (t, c). The per-slot PE stream matches the classic
            # order, but the gin-group matmuls run one slot behind and the
            # head matmul two slots behind, so every cross-engine dependency
            # (sigmoid->q->id_q, tanh->d->e->h'->head) has a full slot of
            # slack and never stalls the TensorEngine or blocks an engine
            # FIFO head-of-line.
            with tc.tile_pool(name="a3p", bufs=3) as a3p, \
                 tc.tile_pool(name="gp", bufs=4) as gp, \
                 tc.tile_pool(name="prp", bufs=1, space="PSUM") as prp, \
                 tc.tile_pool(name="pzp", bufs=1, space="PSUM") as pzp, \
                 tc.tile_pool(name="phngin", bufs=2, space="PSUM") as phngin, \
                 tc.tile_pool(name="phd", bufs=1, space="PSUM") as phd:
                psum_hd = [
                    phd.tile([m_head, F], F32, name=f"pshd{c}", tag=f"pshd{c}")
                    for c in range(NF)
                ]
                NS_ = t_steps * NF
                ctx = {}
                a3_t = None

                def front(s):
                    t, c = divmod(s, NF)
                    nonlocal a3_t
                    if t % A3G == 0 and c == 0:
                        a3_t = a3p.tile([3, A3G * BC], BF16, name="a3t", tag="a3t")
                        [nc.sync, nc.scalar][(t // A3G) % 2].dma_start(
                            a3_t[:], a3_d[:, t * BC:(t + A3G) * BC])
                    hcur = hA if t % 2 == 0 else hB
                    hnxt = hB if t % 2 == 0 else hA
                    cs = slice(c * F, (c + 1) * F)
                    toff = (t % A3G) * BC
                    a3s = slice(toff + c * F, toff + (c + 1) * F)
                    hs = hcur[:, cs]
                    psum_r = prp.tile([128, F], F32, name="psr", tag="psr")
                    psum_z = pzp.tile([128, F], F32, name="psz", tag="psz")
                    rz = gp.tile([128, 2 * F], GD, name="rz", tag="rz")
                    nc.tensor.matmul(psum_r[:], whht_t[:, 0:H], hs,
                                     start=True, stop=False)
                    nc.tensor.matmul(psum_r[:], k3rz_t[:, 0:H],
                                     a3_t[:, a3s], start=False, stop=False)
                    nc.tensor.matmul(psum_r[:], ident[:], gi_r[:, cs],
                                     start=False, stop=True)
                    nc.tensor.matmul(psum_z[:], whht_t[:, H:2 * H], hs,
                                     start=True, stop=False)
                    nc.tensor.matmul(psum_z[:], k3rz_t[:, H:2 * H],
                                     a3_t[:, a3s], start=False, stop=False)
                    nc.tensor.matmul(psum_z[:], ident[:], gi_z[:, cs],
                                     start=False, stop=True)
                    nc.scalar.activation(rz[:, 0:F], psum_r[:], AF.Sigmoid)
                    nc.scalar.activation(rz[:, F:2 * F], psum_z[:], AF.Sigmoid)
                    psum_hn = phngin.tile([128, F], F32, name="pshngin",
                                          tag="pshngin")
                    nc.tensor.matmul(psum_hn[:], whht_t[:, 2 * H:3 * H], hs,
                                     start=True, stop=True)
                    q = gp.tile([128, F], RD, name="q", tag="q")
                    nc.vector.scalar_tensor_tensor(
                        q[:], psum_hn[:], bhhn_t[:], rz[:, 0:F],
                        op0=OP.add, op1=OP.mult,
                    )
                    # u = z*h depends only on the sigmoid -> runs on GPSIMD
                    # in parallel with the q/tanh path
                    u = gp.tile([128, F], GD, name="u", tag="u")
                    nc.gpsimd.tensor_tensor(u[:], rz[:, F:2 * F], hs, op=OP.mult)
                    ctx[s] = (cs, a3s, hs, a3_t, rz, q, hnxt, psum_hn, u)

                def mid(s):
                    t, c = divmod(s, NF)
                    cs, a3s, hs, a3t_s, rz, q, hnxt, psum_gin, u = ctx[s]
                    nc.tensor.matmul(psum_gin[:], ident[:], gi_n[:, cs],
                                     start=True, stop=False)
                    nc.tensor.matmul(psum_gin[:], k2n_t[:], a3t_s[0:2, a3s],
                                     start=False, stop=False)
                    nc.tensor.matmul(psum_gin[:], ident[:], q[:],
                                     start=False, stop=True)
                    nt = gp.tile([128, F], GD, name="nt", tag="nt")
                    nc.scalar.activation(nt[:], psum_gin[:], AF.Tanh)
                    # h' = z*h - (z-1)*n  (u = z*h precomputed on GPSIMD)
                    v = gp.tile([128, F], GD, name="v", tag="v")
                    nc.vector.scalar_tensor_tensor(
                        v[:], rz[:, F:2 * F], 1.0, nt[:],
                        op0=OP.subtract, op1=OP.mult)
                    nc.vector.tensor_tensor(hnxt[:, cs], u[:], v[:],
                                            op=OP.subtract)

                def head(s):
                    t, c = divmod(s, NF)
                    cs = ctx[s][0]
                    hnxt = ctx[s][6]
                    nc.tensor.matmul(
                        psum_hd[c][:],
                        wmsx_t[:, t * m_head:(t + 1) * m_head],
                        hnxt[:, cs],
                        start=(t == 0), stop=(t == t_steps - 1),
                        skip_group_check=True,
                    )
                    del ctx[s]

                # tail drains are emitted inline: as soon as chunk c's last
                # head matmul lands, its psum is copied out, activated, and
                # DMA'd while the remaining chunks still run.
                mu_sb = pp.tile([2 * t_steps, BC], F32)
                std_sb = pp.tile([2 * t_steps, BC], F32)
                scr = gp.tile([1, 1], F32, name="scr", tag="scr")

                def drain(c):
                    fs = slice(c * F, (c + 1) * F)
                    nc.vector.tensor_copy(hh[:, fs], psum_hd[c][:])
                    nc.vector.tensor_scalar_add(mu_sb[:, fs],
                                                hh[0:2 * t_steps, fs], bmu_t[:])
                    nc.scalar.activation(
                        std_sb[:, fs], hh[std_off:std_off + 2 * t_steps, fs],
                        AF.Exp, bias=bstd_t[:], scale=0.5)
                    [nc.sync, nc.scalar][c % 2].dma_start(omu_d[:, fs],
                                                          mu_sb[:, fs])
                    [nc.sync, nc.scalar][c % 2].dma_start(ostd_d[:, fs],
                                                          std_sb[:, fs])

                for s in range(NS_ + 3):
                    if s < NS_:
                        front(s)
                    if s == NS_ - 1:
                        # preload the exp table before the first drain needs it
                        nc.scalar.activation(scr[:], scr[:], AF.Exp)
                    if 0 <= s - 1 < NS_:
                        mid(s - 1)
                    if 0 <= s - 3 < NS_:
                        head(s - 3)
                        if s - 3 >= NS_ - NF:
                            drain((s - 3) % NF)

    nc.compile()
    return nc


_NC_CACHE = {}


def _get_nc(debug=False):
    if "nc" not in _NC_CACHE:
        _NC_CACHE["nc"] = build_nc(debug=debug)
    return _NC_CACHE["nc"]


def make_in_maps(last_obs_state, enc_h_feat, z, sg, fut_traj,
                 W_dh, b_dh, W_vel, b_vel, W_ih, b_ih, W_hh, b_hh,
                 W_mu, b_mu, W_std, b_std, t_steps=T):
    f32 = np.float32
    bf = ml_dtypes.bfloat16

    # ---- weight packing (core-independent) ----
    # W_big: (KIN, 512) ; out cols = [gi_r, gi_z, gi_n, h0]
    wbig = np.zeros((KIN, 512), f32)
    wbig[0:1056, 0:384] = W_ih[:, 0:1056].T
    wbig[0:1056, 384:512] = W_dh.T
    # sg rows: rel = (sg - lo[:, :2])/dt feeds W_ih[:, 1058:1060]
    wbig[1056:1058, 0:384] = (W_ih[:, 1058:1060] / DT_CONST).T
    # lo rows (6): first two carry -W_rel/dt
    wbig[1058:1060, 0:384] = (-W_ih[:, 1058:1060] / DT_CONST).T
    # ones row: input-side biases
    wbig[1064, 0:384] = b_ih
    wbig[1064, 384:512] = b_dh

    whht = np.ascontiguousarray(W_hh.T).astype(f32)          # (128, 384)
    k3rz = np.zeros((3, 2 * H), f32)
    k3rz[0:2, 0:H] = W_ih[0:128, 1056:1058].T                # a -> r gate
    k3rz[2, 0:H] = b_hh[0:128]
    k3rz[0:2, H:2 * H] = W_ih[128:256, 1056:1058].T          # a -> z gate
    k3rz[2, H:2 * H] = b_hh[128:256]
    k2n = np.ascontiguousarray(W_ih[256:384, 1056:1058].T).astype(bf)  # (2,128)
    # head lhsT variants: variant t scatters W_mu/W_std columns to output
    # rows {t, T+t, 2T+t, 3T+t}
    std_off = ((2 * t_steps + 31) // 32) * 32
    m_head = std_off + 2 * t_steps
    wmsx = np.zeros((H, t_steps, m_head), f32)
    for t in range(t_steps):
        wmsx[:, t, t] = W_mu[0]
        wmsx[:, t, t_steps + t] = W_mu[1]
        wmsx[:, t, std_off + t] = W_std[0]
        wmsx[:, t, std_off + t_steps + t] = W_std[1]
    wmsx = wmsx.reshape(H, t_steps * m_head)
    bhhn = b_hh[256:384].reshape(H, 1).astype(f32)
    bmu48 = np.repeat(b_mu, t_steps).reshape(2 * t_steps, 1).astype(f32)
    bstd48 = 0.5 * np.repeat(b_std, t_steps).reshape(2 * t_steps, 1).astype(f32)
    ident128 = np.eye(H, dtype=f32)
    # host-side tiny matmul for a0 (0.4 MFLOP)
    a0 = last_obs_state @ W_vel.T + b_vel                    # (B, 2)

    in_maps = []
    for c in range(NCORES):
        sl = slice(c * BC, (c + 1) * BC)
        xt = np.empty((KIN, BC), f32)
        xt[0:MLP] = enc_h_feat[sl].T
        xt[MLP:1056] = z[sl].T
        xt[1056:1058] = sg[sl].T
        xt[1058:1064] = last_obs_state[sl].T
        xt[1064] = 1.0
        a3 = np.empty((3, t_steps, BC), f32)
        a3[0:2, 0] = a0[sl].T
        for t in range(1, t_steps):
            a3[0:2, t] = fut_traj[t - 1, sl, 2:4].T
        a3[2] = 1.0
        in_maps.append({
            "xt": xt.astype(bf),
            "wbig": wbig.astype(bf),
            "a3": a3.astype(bf),
            "whht": whht,
            "k3rz": k3rz.astype(bf),
            "k2n": k2n,
            "wmsx": wmsx.astype(f32),
            "ident128": ident128,
            "bhhn": bhhn,
            "bmu48": bmu48,
            "bstd48": bstd48,
        })
    return in_maps


def unpack_outputs(results, t_steps=T):
    mus = np.empty((t_steps, B, 2), np.float32)
    stds = np.empty((t_steps, B, 2), np.float32)
    for c in range(NCORES):
        sl = slice(c * BC, (c + 1) * BC)
        omu = results[c]["omu"].reshape(2, t_steps, BC)
        ostd = results[c]["ostd"].reshape(2, t_steps, BC)
        mus[:, sl, 0] = omu[0]
        mus[:, sl, 1] = omu[1]
        stds[:, sl, 0] = ostd[0]
        stds[:, sl, 1] = ostd[1]
    return mus, stds


def kernel(last_obs_state, enc_h_feat, z, sg, fut_traj,
           W_dh, b_dh, W_vel, b_vel, W_ih, b_ih, W_hh, b_hh,
           W_mu, b_mu, W_std, b_std):
    args = dict(
        last_obs_state=np.asarray(last_obs_state, np.float32),
        enc_h_feat=np.asarray(enc_h_feat, np.float32),
        z=np.asarray(z, np.float32),
        sg=np.asarray(sg, np.float32),
        fut_traj=np.asarray(fut_traj, np.float32),
        W_dh=np.asarray(W_dh, np.float32), b_dh=np.asarray(b_dh, np.float32),
        W_vel=np.asarray(W_vel, np.float32), b_vel=np.asarray(b_vel, np.float32),
        W_ih=np.asarray(W_ih, np.float32), b_ih=np.asarray(b_ih, np.float32),
        W_hh=np.asarray(W_hh, np.float32), b_hh=np.asarray(b_hh, np.float32),
        W_mu=np.asarray(W_mu, np.float32), b_mu=np.asarray(b_mu, np.float32),
        W_std=np.asarray(W_std, np.float32), b_std=np.asarray(b_std, np.float32),
    )
    nc = _get_nc()
    in_maps = make_in_maps(**args)
    res = run_bass_kernel_spmd(nc, in_maps, core_ids=list(range(NCORES)))
    return unpack_outputs(res.results)
